# revision 1
# baseline (speedup 1.0000x reference)
"""Trainium2 Bass kernel for nn_RelPosRFFBias — factorized Fourier rewrite, v4 (symmetric triangle).

Same math as v2 (see kernel_v2.py docstring): per head
  bias = Q + sign(ct-cs) * P
with Q/P rank-~247 cosine-series matmuls in sorted-center order, so off-diag
128x128 tiles fold +-P straight into Q's PSUM accumulation and only diagonal
tiles need a DVE sign-multiply.

v3 schedule fixes (from the v2 trace):
 - rhs builds are emitted just-in-time per head pair, interleaved with the
   main loop, so the DVE FIFO doesn't delay diag overrides / output DMAs.
 - diag-P matmuls merged 4-heads-per-bank via 3D rhs APs into a contiguous
   per-chunk rhs tile (32 full 512-free MMs instead of 128 at 128-free whose
   LDWEIGHTS serialized).
 - input DMAs ordered so build-feeding tables land first; output DMA of the
   last row-tile split into quarters to shrink the tail.
"""

import math

import numpy as np

B, T = 8, 512
RFF, NH = 16, 16
F_MIN, F_MAX = 2.0, 64.0
TWO_PI = 2.0 * math.pi

N_CORES = 8
L_PER = 1.0625
KU = 124
NQMAX = 128
TILE = 128
GRP = 2
NT = T // TILE
FIT_LAM = 1e-5
FIT_ITERS = 14

_MODULE = None
_LAST_RESULTS = None
_FIT_CACHE = {}


# ---------------------------------------------------------------- host: fit
def _gelu64(x):
    try:
        from scipy.special import erf
    except ImportError:
        erf = np.vectorize(math.erf)
    return 0.5 * x * (1.0 + erf(x / math.sqrt(2.0)))


def _g_of_D(D, phase, W1, b1, W2, b2, freqs):
    arg = TWO_PI * D[:, None] * freqs[None, :] + phase[None, :]
    feats = np.concatenate([np.sin(arg), np.cos(arg)], axis=-1)
    return _gelu64(feats @ W1 + b1) @ W2 + b2


def _tone_grid():
    freqs = np.logspace(math.log10(F_MIN), math.log10(F_MAX), RFF).astype(np.float64)
    uni = np.arange(KU) / L_PER
    cut = uni[-1]
    cand = sorted(set(
        round(f, 6)
        for f in np.concatenate([(freqs[:, None] + freqs[None, :]).ravel(), 2 * freqs])
        if cut + 0.2 < f < 145.0
    ))
    omQ = np.concatenate([uni, np.asarray(cand[: NQMAX - KU], dtype=np.float64)])
    omQ.sort()
    omP = omQ[1:min(len(omQ), 129)]
    return freqs, omQ, omP


def _fit_coefs(phase, W1, b1, W2, b2):
    freqs, omQ, omP = _tone_grid()
    NG = 32768
    Dg = (np.arange(NG) + 0.5) / NG
    G = _g_of_D(Dg, phase, W1, b1, W2, b2, freqs)
    Phi = np.concatenate(
        [np.cos(Dg[:, None] * TWO_PI * omQ[None, :]),
         np.sin(Dg[:, None] * TWO_PI * omP[None, :])], axis=1)
    lam = FIT_LAM * NG
    w = np.ones(NG)
    best = None
    for _ in range(FIT_ITERS):
        Pw = Phi * w[:, None]
        A = Pw.T @ Phi
        A[np.diag_indices_from(A)] += lam
        coef = np.linalg.solve(A, Pw.T @ G)
        res = np.abs(Phi @ coef - G).max(axis=1)
        mx = res.max()
        if best is None or mx < best[0]:
            best = (mx, coef.copy())
        w = w * (0.05 + res / mx)
        w = np.maximum(w / w.mean(), 1e-6)
    mx, coef = best
    return omQ, omP, coef[: len(omQ)], coef[len(omQ):], mx


# ---------------------------------------------------------------- device
def _build_module():
    import concourse.tile as tile
    from concourse import bacc, mybir
    from contextlib import ExitStack

    f32 = mybir.dt.float32
    bf16 = mybir.dt.bfloat16
    Alu = mybir.AluOpType
    Act = mybir.ActivationFunctionType

    nc = bacc.Bacc("TRN2", target_bir_lowering=False, debug=False)

    # packed inputs: A = [ups0|ups1|uq0|uq1] bf16, B = [up0|up1] bf16,
    # F = [s4 | aq | bp] f32
    packA_d = nc.dram_tensor("packA", [TILE, 4 * T], bf16, kind="ExternalInput")
    packB_d = nc.dram_tensor("packB", [TILE, 2 * T], bf16, kind="ExternalInput")
    packR_d = nc.dram_tensor("packR", [2 * TILE, 4 * T], bf16,
                             kind="ExternalInput")
    packF_d = nc.dram_tensor("packF", [TILE, 4 * TILE + 4 * NH], f32,
                             kind="ExternalInput")
    # compact block-lower-triangle outputs, one per row-tile i
    out_ds = [nc.dram_tensor(f"out{i}", [TILE, NH * (i + 1) * TILE], bf16,
                             kind="ExternalOutput") for i in range(NT)]

    NG_ = NH // GRP

    with tile.TileContext(nc) as tc:
        with ExitStack() as ctx:
            const = ctx.enter_context(tc.tile_pool(name="const", bufs=1))
            rhspool = ctx.enter_context(tc.tile_pool(name="rhs", bufs=1))
            xpool = ctx.enter_context(tc.tile_pool(name="xt", bufs=3))
            stpool = ctx.enter_context(tc.tile_pool(name="stage", bufs=2))
            pmain = ctx.enter_context(tc.tile_pool(name="pmain", bufs=3, space="PSUM"))
            ppd = ctx.enter_context(tc.tile_pool(name="ppd", bufs=2, space="PSUM"))

            # packed input tiles (3 DMAs; builds need A+F only)
            packA = const.tile([TILE, 4 * T], bf16, tag="packA")
            nc.sync.dma_start(packA[:], packA_d.ap())
            packF = const.tile([TILE, 4 * TILE + 4 * NH], f32, tag="packF")
            nc.scalar.dma_start(packF[:], packF_d.ap())
            packB = const.tile([TILE, 2 * T], bf16, tag="packB")
            nc.sync.dma_start(packB[:], packB_d.ap())
            ups_t = [packA[:, c * T:(c + 1) * T] for c in range(2)]
            uq_t = [packA[:, (2 + c) * T:(3 + c) * T] for c in range(2)]
            up_t = [packB[:, c * T:(c + 1) * T] for c in range(2)]
            s4_t = packF[:, 0:4 * TILE]
            aq_t = packF[:, 4 * TILE:4 * TILE + 2 * NH]
            bp_t = packF[:, 4 * TILE + 2 * NH:4 * TILE + 4 * NH]

            # PE warm-up: dummy matmuls while input DMAs land, so HAM is
            # at K=8/8 before the real MM stream starts.
            warm_sb = const.tile([TILE, TILE], bf16, tag="warm")
            nc.vector.memset(warm_sb[:], 0)
            warm_ps = ppd.tile([TILE, TILE], f32, tag="pd", bufs=2)
            for _ in range(100):
                nc.tensor.matmul(warm_ps[:], warm_sb[:], warm_sb[:],
                                 start=True, stop=True)

            # big contiguous rhs tiles: [128, NH*T] per chunk, head-major
            rq_big = [rhspool.tile([TILE, NH * T], bf16, tag=f"rqb{c}",
                                   name=f"rqb{c}") for c in range(2)]
            rp_big = [rhspool.tile([TILE, NH * T], bf16, tag=f"rpb{c}",
                                   name=f"rpb{c}") for c in range(2)]
            for c in range(2):
                nc.sync.dma_start(rp_big[c][:, 0:4 * T],
                                  packR_d.ap()[c * TILE:(c + 1) * TILE])
            built = [False] * NH

            def ensure_built(h):
                if h >= NH or built[h]:
                    return
                built[h] = True
                for c in range(2):
                    col = 2 * h + c
                    nc.vector.tensor_scalar(
                        rq_big[c][:, h * T:(h + 1) * T], uq_t[c],
                        aq_t[:, col:col + 1], None, Alu.mult)
                    if h < 4:
                        continue        # rp host-baked via packR
                    if c == 1:
                        nc.scalar.activation(
                            rp_big[c][:, h * T:(h + 1) * T], ups_t[c],
                            Act.Identity, scale=bp_t[:, col:col + 1])
                    else:
                        nc.vector.tensor_scalar(
                            rp_big[c][:, h * T:(h + 1) * T], ups_t[c],
                            bp_t[:, col:col + 1], None, Alu.mult)

            ensure_built(0)
            ensure_built(1)
            ensure_built(2)
            ensure_built(3)

            stages = {}
            pm_hist = {}

            def body(i, g):
                t0 = i * TILE
                Wi = (i + 1) * TILE
                if g == 0:
                    stages[i] = stpool.tile([TILE, NH * Wi], bf16,
                                            tag=f"stage{i}", bufs=1,
                                            name=f"stage{i}")
                    pm_hist[i] = {}
                stage = stages[i]
                h0 = g * GRP
                pm = pmain.tile([TILE, GRP * T], f32, tag="pm")
                for c in range(2):
                    for j in range(GRP):
                        h = h0 + j
                        nc.tensor.matmul(
                            pm[:, j * T:j * T + Wi],
                            uq_t[c][:, t0:t0 + TILE],
                            rq_big[c][:, h * T:h * T + Wi],
                            start=(c == 0), stop=False,
                            skip_group_check=True)
                if i > 0:
                    for c in range(2):
                        for j in range(GRP):
                            h = h0 + j
                            nc.tensor.matmul(
                                pm[:, j * T:j * T + t0],
                                up_t[c][:, t0:t0 + TILE],
                                rp_big[c][:, h * T:h * T + t0],
                                start=False, stop=(c == 1),
                                skip_group_check=True)
                if i > 0:
                    pm_ev = pm[:].rearrange("p (e s) -> p e s", e=GRP)[:, :, 0:t0]
                    st_ev = stage[:].rearrange("p (e s) -> p e s", e=NH)[
                        :, h0:h0 + GRP, 0:t0]
                    nc.scalar.activation(st_ev, pm_ev, Act.Identity)
                if g % 2 == 1:
                    h4 = 2 * (g - 1)
                    pd = ppd.tile([TILE, 4 * TILE], f32, tag="pd")
                    rp_v = [rp_big[c][:].rearrange(
                        "p (e s) -> p e s", e=NH)[:, h4:h4 + 4, t0:t0 + TILE]
                        for c in range(2)]
                    for c in range(2):
                        nc.tensor.matmul(
                            pd[:], up_t[c][:, t0:t0 + TILE], rp_v[c],
                            start=(c == 0), stop=(c == 1))
                    xt = xpool.tile([TILE, 4 * TILE], f32, tag="xt")
                    nc.vector.tensor_tensor(xt[:], pd[:], s4_t, Alu.mult)
                    pm_prev = pm_hist[i][g - 1]
                    for k, pmk in ((0, pm_prev), (2, pm)):
                        st_v = stage[:].rearrange(
                            "p (e s) -> p e s", e=NH)[
                            :, h4 + k:h4 + k + 2, t0:t0 + TILE]
                        pm_v = pmk[:].rearrange(
                            "p (e s) -> p e s", e=GRP)[:, :, t0:t0 + TILE]
                        xt_v = xt[:].rearrange(
                            "p (e s) -> p e s", e=4)[:, k:k + 2, :]
                        nc.vector.tensor_tensor(st_v, pm_v, xt_v, Alu.add)
                pm_hist[i][g] = pm
                ensure_built(h0 + 2)
                ensure_built(h0 + 3)
                o_ap = out_ds[i].ap()
                eng = nc.sync if i >= 1 else nc.scalar
                if g % 2 == 1:
                    q0 = (g // 2) * (NH * Wi // 4)
                    q1 = q0 + NH * Wi // 4
                    eng.dma_start(o_ap[:, q0:q1], stage[:, q0:q1])

            for i in (3, 2):
                for g in range(NG_):
                    body(i, g)
            for g in range(NG_):
                body(1, g)
                body(0, g)

    nc.compile()
    return nc


# ---------------------------------------------------------------- host glue
def _to_bf16(x):
    import ml_dtypes
    return np.ascontiguousarray(x, np.float32).astype(ml_dtypes.bfloat16)


def _host_tables(c_sorted, omQ, omP):
    def interleave(om):
        ang = np.multiply.outer(om, c_sorted.astype(np.float64)) * TWO_PI
        out = np.zeros((2 * TILE, T), np.float32)
        out[0:2 * len(om):2] = np.cos(ang)
        out[1:2 * len(om):2] = np.sin(ang)
        return out

    uq = interleave(omQ)
    up = interleave(omP)
    ups = np.zeros_like(up)
    ups[0::2] = up[1::2]
    ups[1::2] = up[0::2]
    return uq, up, ups


def _coef_cols(a, b, omQ, omP):
    aq = np.zeros((TILE, 2 * NH), np.float32)
    bp = np.zeros((TILE, 2 * NH), np.float32)
    for h in range(NH):
        for c in range(2):
            col = 2 * h + c
            for k in range(TILE // 2):
                kk = c * (TILE // 2) + k
                if kk < len(omQ):
                    aq[2 * k, col] = a[kk, h]
                    aq[2 * k + 1, col] = a[kk, h]
                if kk < len(omP):
                    bp[2 * k, col] = -b[kk, h]
                    bp[2 * k + 1, col] = b[kk, h]
    return aq, bp


def kernel(centers01, mask, bias_phase, W1, b1, W2, b2):
    global _MODULE, _LAST_RESULTS
    from concourse.bass_utils import run_bass_kernel_spmd

    centers01 = np.asarray(centers01, np.float32)
    bias_phase = np.asarray(bias_phase, np.float64)
    W1 = np.asarray(W1, np.float64)
    b1 = np.asarray(b1, np.float64)
    W2 = np.asarray(W2, np.float64)
    b2 = np.asarray(b2, np.float64)

    ck = hash((bias_phase.tobytes(), W1.tobytes(), b1.tobytes(),
               W2.tobytes(), b2.tobytes()))
    if ck not in _FIT_CACHE:
        _FIT_CACHE[ck] = _fit_coefs(bias_phase, W1, b1, W2, b2)
    omQ, omP, a, b, _gridmax = _FIT_CACHE[ck]

    aq, bp = _coef_cols(a, b, omQ, omP)
    s4 = np.tile(np.sign(np.arange(TILE)[:, None] - np.arange(TILE)[None, :])
                 .astype(np.float32), (1, 4))

    if _MODULE is None:
        _MODULE = _build_module()
    nc = _MODULE

    in_maps = []
    idxs = []
    for bi in range(N_CORES):
        c = centers01[bi]
        idx = np.argsort(c, kind="stable")
        idxs.append(idx)
        uq, up, ups = _host_tables(c[idx], omQ, omP)
        packA = np.concatenate(
            [ups[0:TILE], ups[TILE:2 * TILE], uq[0:TILE], uq[TILE:2 * TILE]],
            axis=1)
        packB = np.concatenate([up[0:TILE], up[TILE:2 * TILE]], axis=1)
        packF = np.concatenate([s4, aq, bp], axis=1).astype(np.float32)
        ups16 = _to_bf16(ups).astype(np.float32)
        packR = np.concatenate(
            [np.concatenate(
                [ups16[c * TILE:(c + 1) * TILE] * bp[:, 2 * h + c:2 * h + c + 1]
                 for h in range(4)], axis=1)
             for c in range(2)], axis=0)
        in_maps.append({
            "packA": _to_bf16(packA), "packB": _to_bf16(packB),
            "packR": _to_bf16(packR), "packF": packF,
        })

    res = run_bass_kernel_spmd(nc, in_maps, list(range(N_CORES)))
    _LAST_RESULTS = res

    out = np.empty((B, NH, T, T), np.float32)
    M = np.empty((NH, T, T), np.float32)
    for bi in range(N_CORES):
        for i in range(4):
            raw = np.asarray(res.results[bi][f"out{i}"])
            if raw.dtype != np.uint16:
                raw = raw.view(np.uint16)
            f = (raw.astype(np.uint32) << 16).view(np.float32)
            # [128, NH, (i+1)*128] -> M[:, ti, :cols]
            M[:, i * TILE:(i + 1) * TILE, 0:(i + 1) * TILE] = \
                f.reshape(TILE, NH, (i + 1) * TILE).transpose(1, 0, 2)
        Mb = M.reshape(NH, 4, TILE, 4, TILE)
        for ii in range(4):
            for jj in range(ii + 1, 4):
                Mb[:, ii, :, jj, :] = Mb[:, jj, :, ii, :].swapaxes(-2, -1)
        inv = np.empty(T, np.int64)
        inv[idxs[bi]] = np.arange(T)
        out[bi] = M[:, inv][:, :, inv]
    m = np.asarray(mask, bool)
    if not m.all():
        out *= (m[:, None, :, None] & m[:, None, None, :]).astype(np.float32)
    return out

